# revision 28
# baseline (speedup 1.0000x reference)
"""CQAttention (context-query attention, BiDAF/QANet-style) Trainium2 kernel.

Problem: B=8, Lc=2048, Lq=512, d=512.
  S[b,i,j] = C_i.wc + Q_j.wq + sum_k wm_k C_ik Q_jk + b  (trilinear score)
  Sq = softmax_j(S); Sc = softmax_i(S)
  A  = Sq @ Q;  Bm = Sq @ (Sc^T @ C)
  out = [C | A | C*A | C*Bm]   -> [B, Lc, 4d]

Strategy: data-parallel over batch across the 8 NeuronCores (one batch per
core).  Per core:

  P1: S tile  = (C*wm) @ Q^T + qb aug-row      [128, Lq] PSUM   (f32r matmul)
  P2: E = exp(S + c_i)  (scalar engine; bias per-partition; accum -> rowsum)
  T : E^T via PE transpose (bf16, 1 cyc/row)   -> PSUM bf16
      scalar Copy PSUM->SBUF assembles Et[j] (accum -> colsum partials)
  P6: Abar = E @ Q      (lhsT = Et cols)   A = Abar * 1/rowsum
  P5: F = E^T @ C       (lhsT = En cols)   ScTC = F * 1/colsum
  P7: Bmbar = E @ ScTC  (lhsT = Et cols)   Bm = Bmbar * 1/rowsum

vs the previous version this drops the *recomputed* transposed-score matmul
(40960 PE cycles) in favor of 64 PE transposes of E (8192 cycles); the
transposed exp pass becomes a same-cost scalar Copy.  Output is written
bf16 (host upcasts) halving out-DMA to 8.4 MB; the redundant C f32 input
load is dropped (block0 passthrough + C*A / C*Bm read the bf16 copy).

Elementwise tail work is spread over three engines (A on vector, Bm-scale
on scalar, C*Bm on gpsimd) so the P7 tail stays PE-bound.

Host side precomputes cheap O(L*d) vectors and layout transposes:
  wc/wq/wm split, c = C@wc (col-bias), qb = Q@wq + bias (aug row),
  CT = C^T (f32r), QmT = (Q*wm)^T (f32r), Cbf/Qbf = bf16 casts.
"""

import numpy as np

_B, _LC, _LQ, _D = 8, 2048, 512, 512
_P = 128


def _ensure_import():
    try:
        import concourse.bass  # noqa: F401
    except ImportError:
        import sys

        for p in ("/opt/trn_rl_repo", "/root/.axon_site/_ro/trn_rl_repo"):
            if p not in sys.path:
                sys.path.insert(0, p)
        import concourse.bass  # noqa: F401


def build_program(Lc=_LC, Lq=_LQ, D=_D):
    """Build the single-core Bass program (identical across the 8 cores)."""
    _ensure_import()
    from contextlib import ExitStack

    import concourse.mybir as mybir
    from concourse import bacc
    from concourse import masks
    from concourse.tile import TileContext

    f32 = mybir.dt.float32
    f32r = mybir.dt.float32r
    bf16 = mybir.dt.bfloat16
    EXP = mybir.ActivationFunctionType.Exp
    AXX = mybir.AxisListType.X
    P = _P
    NLc, NLq, ND = Lc // P, Lq // P, D // P
    CHUNK = 512
    NCH = Lc // CHUNK
    WT = 4  # row-tiles per transpose window
    NW = NLc // WT

    nc = bacc.Bacc()
    dCT = nc.declare_dram_parameter("CTp", [P, NCH * ND * CHUNK], bf16, isOutput=False)
    dQmT = nc.declare_dram_parameter("QmTp", [P, ND * Lq], bf16, isOutput=False)
    dCbf = nc.declare_dram_parameter("Cbf", [Lc, D], bf16, isOutput=False)
    dQbf = nc.declare_dram_parameter("Qbfp", [P, NLq * D], bf16, isOutput=False)
    dccols = nc.declare_dram_parameter("c_cols", [P, NLc], f32, isOutput=False)
    dsm = nc.declare_dram_parameter("smalls", [1, Lq + P], f32r, isOutput=False)
    dout = nc.declare_dram_parameter("out", [Lc, 4 * D], bf16, isOutput=True)

    with ExitStack() as ctx:
        tc = ctx.enter_context(TileContext(nc))
        sb = ctx.enter_context(tc.tile_pool(name="persist", bufs=1))
        psum = ctx.enter_context(tc.tile_pool(name="psum", bufs=1, space="PSUM"))
        stage = ctx.enter_context(tc.tile_pool(name="stage", bufs=4))

        # ---- persistent SBUF tiles ----
        tCTn = [
            sb.tile([P, ND * CHUNK], bf16, tag=f"CTn{n}", name=f"CTn{n}")
            for n in range(NCH)
        ]
        tQmTs = sb.tile([P, ND * Lq], bf16, name="QmTs")
        tQmT = [tQmTs[:, k * Lq : (k + 1) * Lq] for k in range(ND)]
        tCb = [sb.tile([P, D], bf16, tag=f"Cb{i}", name=f"Cb{i}") for i in range(NLc)]
        tQs = sb.tile([P, NLq * D], bf16, name="Qs")
        tQ = [tQs[:, j * D : (j + 1) * D] for j in range(NLq)]
        tEn = [sb.tile([P, Lq], bf16, tag=f"En{i}", name=f"En{i}") for i in range(NLc)]
        tEt = [sb.tile([P, Lc], bf16, tag=f"Et{j}", name=f"Et{j}") for j in range(NLq)]
        tSc = [sb.tile([P, D], bf16, tag=f"Sc{j}", name=f"Sc{j}") for j in range(NLq)]
        tcb = sb.tile([P, NLc], f32, name="cbias")
        tsm = sb.tile([1, Lq + P], f32r, name="tsm")
        tqrow = tsm[:, 0:Lq]
        tones = tsm[:, Lq : Lq + P]
        tident = sb.tile([P, P], bf16, name="ident")
        trs0 = [sb.tile([P, 1], f32, tag=f"rs0{i}", name=f"rs0{i}") for i in range(NLc)]
        trsr = [sb.tile([P, 1], f32, tag=f"rsr{i}", name=f"rsr{i}") for i in range(NLc)]
        tcsp = [sb.tile([P, NW], f32, tag=f"csp{j}", name=f"csp{j}") for j in range(NLq)]
        tcs0 = [sb.tile([P, 1], f32, tag=f"cs0{j}", name=f"cs0{j}") for j in range(NLq)]
        tcsr = [sb.tile([P, 1], f32, tag=f"csr{j}", name=f"csr{j}") for j in range(NLq)]

        twarm = sb.tile([P, Lq], bf16, name="twarm")

        # ---- input DMA (ordered by first-consumer time) ----
        # P1(0) needs CT[k][0]+QmT[k] in k order -- those go absolutely
        # first so the PE starts ASAP (each descriptor has ~0.7us latency).
        nc.gpsimd.memset(twarm[:], 0.25)
        masks.make_identity(nc, tident[:])
        # dummy exp: loads the scalar engine's activation table during the
        # DMA head instead of stalling the first real P2
        nc.scalar.activation(tcs0[0][:], twarm[:, 0:1], EXP)
        CW = ND * CHUNK
        nc.sync.dma_start(out=tCTn[0][:], in_=dCT[:, 0:CW])
        nc.sync.dma_start(out=tQmTs[:], in_=dQmT[:, :])
        nc.sync.dma_start(out=tsm[:], in_=dsm[:, :])
        nc.sync.dma_start(out=tcb[:], in_=dccols[:, :])
        nc.sync.dma_start(out=tCTn[1][:], in_=dCT[:, CW : 2 * CW])
        # Q (needed by P6 of window 0), then first window's Cbf, then the
        # rest of CT paced ahead of the P1 consumer, then remaining Cbf.
        nc.sync.dma_start(out=tQs[:], in_=dQbf[:, :])
        for i in range(WT):
            nc.sync.dma_start(out=tCb[i][:], in_=dCbf[i * P : (i + 1) * P, :])
        nc.sync.dma_start(out=tCTn[2][:], in_=dCT[:, 2 * CW : 3 * CW])
        for i in range(WT, 2 * WT):
            nc.sync.dma_start(out=tCb[i][:], in_=dCbf[i * P : (i + 1) * P, :])
        nc.sync.dma_start(out=tCTn[3][:], in_=dCT[:, 3 * CW : 4 * CW])
        for i in range(2 * WT, NLc):
            nc.sync.dma_start(out=tCb[i][:], in_=dCbf[i * P : (i + 1) * P, :])

        # ---- PE warmup: full-K (128-row) matmuls on a memset tile (no DMA
        # dependency, starts during the input-DMA head).  Real array
        # activity for the HAM clock-gate; sized to end about when P1(0)'s
        # operands have landed so P1 is never delayed.
        warm_ps = psum.tile([P, Lq], f32, tag="psA", name="warm_ps", bufs=2)
        for _w in range(12):
            nc.tensor.matmul(
                warm_ps[:], twarm[:, 0:P], twarm[:], start=True, stop=True
            )

        # ---- main pipeline ----
        # per step s: P1/P2 for tile s; transposes for tile s-1 lag one step
        # behind so the scalar exp has a P1 of slack.  When a window's last
        # tile is transposed, the vector engine flushes the psT pair into
        # Et[j] (tensor_tensor_reduce bypass; accum -> colsum partials) and
        # P6 for that window runs one step later, giving the copies a P1+T
        # of slack.  Elementwise: A-scale on gpsimd, C*A on vector.
        cur_psT = None
        MUL = mybir.AluOpType.mult
        ADD = mybir.AluOpType.add

        def emit_copies_recips(w):
            # copies first: they gate P6 (PE); recips only gate A-scales
            for j in range(NLq):
                src = cur_psT[j // 2][:, (j % 2) * (WT * P) : (j % 2 + 1) * (WT * P)]
                nc.vector.tensor_scalar(
                    out=tEt[j][:, w * (WT * P) : (w + 1) * (WT * P)],
                    in0=src,
                    scalar1=1.0,
                    scalar2=None,
                    op0=MUL,
                    op1=ADD,
                    accum_out=tcsp[j][:, w : w + 1],
                )
            for i in range(w * WT, (w + 1) * WT):
                nc.vector.reciprocal(trsr[i][:], trs0[i][:])

        def emit_P6_tile(i):
            psA = psum.tile([P, D], f32, tag="psA", name=f"psa{i}", bufs=2)
            for j in range(NLq):
                nc.tensor.matmul(
                    psA[:],
                    tEt[j][:, i * P : (i + 1) * P],
                    tQ[j],
                    start=(j == 0),
                    stop=(j == NLq - 1),
                )
            tA = stage.tile([P, D], bf16, tag="A", name=f"A{i}")
            nc.vector.tensor_scalar_mul(tA[:], psA[:], trsr[i][:])
            tCA = stage.tile([P, D], bf16, tag="CA", name=f"CA{i}")
            nc.gpsimd.tensor_mul(tCA[:], tCb[i][:], tA[:])
            nc.sync.dma_start(out=dout[i * P : (i + 1) * P, 0:D], in_=tCb[i][:])
            nc.sync.dma_start(out=dout[i * P : (i + 1) * P, D : 2 * D], in_=tA[:])
            nc.sync.dma_start(out=dout[i * P : (i + 1) * P, 2 * D : 3 * D], in_=tCA[:])

        psF = [None] * NLq

        def emit_P5_mms(j):
            psF[j] = psum.tile([P, D], f32, tag="ps", name=f"psf{j}", bufs=2)
            for k in range(NLc):
                nc.tensor.matmul(
                    psF[j][:],
                    tEn[k][:, j * P : (j + 1) * P],
                    tCb[k][:],
                    start=(k == 0),
                    stop=(k == NLc - 1),
                )

        def emit_P5_scale(j):
            if j == 0:
                for jj in range(NLq):
                    nc.vector.reduce_sum(tcs0[jj][:], tcsp[jj][:], axis=AXX)
                    nc.vector.reciprocal(tcsr[jj][:], tcs0[jj][:])
            nc.vector.tensor_scalar_mul(tSc[j][:], psF[j][:], tcsr[j][:])

        for s in range(NLc + 5):
            if s < NLc:
                ps = psum.tile([P, Lq], f32, tag="ps", name=f"psn{s}", bufs=2)
                for k in range(ND):
                    nc.tensor.matmul(
                        ps[:],
                        tCTn[s // 4][
                            :, k * CHUNK + (s % 4) * P : k * CHUNK + (s % 4 + 1) * P
                        ],
                        tQmT[k],
                        start=(k == 0),
                        stop=False,
                    )
                nc.tensor.matmul(ps[:], tones, tqrow, start=False, stop=True)
                nc.scalar.activation(
                    tEn[s][:], ps[:], EXP, bias=tcb[:, s : s + 1], accum_out=trs0[s][:]
                )
            if s == NLc:
                # P5(j0) fills the PE between the last P1 and T(15): its
                # early k-matmuls depend only on long-finished exp outputs,
                # giving exp(15) time to land before T(15) consumes it.
                emit_P5_mms(0)
            if 1 <= s <= NLc:
                i = s - 1
                if i % WT == 0:
                    cur_psT = [
                        psum.tile(
                            [P, 2 * WT * P],
                            bf16,
                            tag="psT",
                            name=f"psT{i // WT}_{pr}",
                            bufs=4,
                        )
                        for pr in range(2)
                    ]
                for j in range(NLq):
                    nc.tensor.transpose(
                        cur_psT[j // 2][
                            :,
                            (j % 2) * (WT * P)
                            + (i % WT) * P : (j % 2) * (WT * P)
                            + (i % WT + 1) * P,
                        ],
                        tEn[i][:, j * P : (j + 1) * P],
                        tident[:],
                    )
                if i % WT == WT - 1:
                    emit_copies_recips(i // WT)
            # tail: P5(j) interleaved with the remaining P6 tiles so the
            # vector-side colsum finalize + Sc scales hide under PE matmuls
            if NLc + 1 <= s <= NLc + 3:
                j = s - NLc
                emit_P5_scale(j - 1)
                emit_P5_mms(j)
            # P6 spread one tile per step: tile s-5 (its window's copies are
            # emitted at step 4w+4 <= s-1, so the vector flush has slack)
            if 5 <= s < NLc + 5:
                emit_P6_tile(s - 5)
        emit_P5_scale(NLq - 1)

        # ---- P7: Bmbar per row-tile -> Bm (scalar), C*Bm (gpsimd) ----
        for i in range(NLc):
            psB = psum.tile([P, D], f32, tag="psA", name=f"psb{i}", bufs=2)
            for j in range(NLq):
                nc.tensor.matmul(
                    psB[:],
                    tEt[j][:, i * P : (i + 1) * P],
                    tSc[j][:],
                    start=(j == 0),
                    stop=(j == NLq - 1),
                )
            tBm = stage.tile([P, D], bf16, tag="BM", name=f"Bm{i}")
            nc.scalar.mul(tBm[:], psB[:], trsr[i][:])
            tCB = stage.tile([P, D], bf16, tag="CB", name=f"CB{i}")
            nc.vector.tensor_mul(tCB[:], tCb[i][:], tBm[:])
            nc.sync.dma_start(out=dout[i * P : (i + 1) * P, 3 * D : 4 * D], in_=tCB[:])

    nc.finalize()
    return nc


def round_fp32r(a):
    """Round fp32 to the fp32r encoding: RNE to 11 mantissa bits, low 12
    bits zero.  Matmul operands must carry this encoding (the PE consumes
    the top 20 bits)."""
    a = np.ascontiguousarray(a, np.float32)
    u = a.view(np.uint32)
    u = (u + 0x7FF + ((u >> 12) & 1)) & np.uint32(0xFFFFF000)
    return u.view(np.float32)


def pack_rows(a):
    """[R*128, N] -> [128, R*N]: 128-row bands concatenated along columns."""
    R = a.shape[0] // _P
    return np.ascontiguousarray(
        np.concatenate([a[r * _P : (r + 1) * _P, :] for r in range(R)], axis=1)
    )


def pack_ct(ct):
    """CT [D, Lc] -> [128, NCH*ND*CHUNK], chunk-major then k-band."""
    D, Lc = ct.shape
    CHUNK = 512
    blocks = []
    for n in range(Lc // CHUNK):
        for k in range(D // _P):
            blocks.append(ct[k * _P : (k + 1) * _P, n * CHUNK : (n + 1) * CHUNK])
    return np.ascontiguousarray(np.concatenate(blocks, axis=1))


def prepare_in_maps(C, Q, Wo_w, Wo_b):
    """Shard over batch; per batch precompute layouts + rank-1 vectors."""
    import ml_dtypes

    D = C.shape[-1]
    P = _P
    w = np.asarray(Wo_w, np.float32)[0]
    wc, wq, wm = w[:D], w[D : 2 * D], w[2 * D :]
    b0 = np.float32(np.asarray(Wo_b, np.float32)[0])
    in_maps = []
    for b in range(C.shape[0]):
        Cb = np.ascontiguousarray(C[b], np.float32)
        Qb = np.ascontiguousarray(Q[b], np.float32)
        cvec = (Cb @ wc).astype(np.float32)
        qbvec = (Qb @ wq + b0).astype(np.float32)
        in_maps.append(
            {
                "CTp": pack_ct(Cb.T.astype(ml_dtypes.bfloat16)),
                "Cbf": Cb.astype(ml_dtypes.bfloat16),
                "Qbfp": pack_rows(Qb.astype(ml_dtypes.bfloat16)),
                "QmTp": pack_rows((Qb * wm).T.astype(ml_dtypes.bfloat16)),
                "c_cols": np.ascontiguousarray(cvec.reshape(-1, _P).T),
                "smalls": round_fp32r(
                    np.concatenate([qbvec, np.ones(_P, np.float32)])[None, :]
                ),
            }
        )
    return in_maps


_prog_cache = {}


def _get_program():
    if "nc" not in _prog_cache:
        _prog_cache["nc"] = build_program()
    return _prog_cache["nc"]


def run(C, Q, Wo_w, Wo_b, **spmd_kwargs):
    """Run on hardware; returns (out [B,Lc,4d] f32, BassKernelResults)."""
    _ensure_import()
    from concourse.bass_utils import run_bass_kernel_spmd

    nc = _get_program()
    in_maps = prepare_in_maps(C, Q, Wo_w, Wo_b)
    res = run_bass_kernel_spmd(nc, in_maps, list(range(len(in_maps))), **spmd_kwargs)
    out = np.stack(
        [np.asarray(res.results[i]["out"], np.float32) for i in range(len(in_maps))],
        axis=0,
    )
    return out, res


def kernel(C, Q, Wo_w, Wo_b):
    out, _ = run(C, Q, Wo_w, Wo_b)
    return out


# revision 34
# speedup vs baseline: 1.1048x; 1.1048x over previous
"""CQAttention (context-query attention, BiDAF/QANet-style) Trainium2 kernel.

Problem: B=8, Lc=2048, Lq=512, d=512.
  S[b,i,j] = C_i.wc + Q_j.wq + sum_k wm_k C_ik Q_jk + b  (trilinear score)
  Sq = softmax_j(S); Sc = softmax_i(S)
  A  = Sq @ Q;  Bm = Sq @ (Sc^T @ C)
  out = [C | A | C*A | C*Bm]   -> [B, Lc, 4d]

Strategy: data-parallel over batch across the 8 NeuronCores (one batch per
core).  Per core:

  P1: S tile  = (C*wm) @ Q^T + qb aug-row      [128, Lq] PSUM   (f32r matmul)
  P2: E = exp(S + c_i)  (scalar engine; bias per-partition; accum -> rowsum)
  T : E^T via PE transpose (bf16, 1 cyc/row)   -> PSUM bf16
      scalar Copy PSUM->SBUF assembles Et[j] (accum -> colsum partials)
  P6: Abar = E @ Q      (lhsT = Et cols)   A = Abar * 1/rowsum
  P5: F = E^T @ C       (lhsT = En cols)   ScTC = F * 1/colsum
  P7: Bmbar = E @ ScTC  (lhsT = Et cols)   Bm = Bmbar * 1/rowsum

vs the previous version this drops the *recomputed* transposed-score matmul
(40960 PE cycles) in favor of 64 PE transposes of E (8192 cycles); the
transposed exp pass becomes a same-cost scalar Copy.  Output is written
bf16 (host upcasts) halving out-DMA to 8.4 MB; the redundant C f32 input
load is dropped (block0 passthrough + C*A / C*Bm read the bf16 copy).

Elementwise tail work is spread over three engines (A on vector, Bm-scale
on scalar, C*Bm on gpsimd) so the P7 tail stays PE-bound.

Host side precomputes cheap O(L*d) vectors and layout transposes:
  wc/wq/wm split, c = C@wc (col-bias), qb = Q@wq + bias (aug row),
  CT = C^T (f32r), QmT = (Q*wm)^T (f32r), Cbf/Qbf = bf16 casts.
"""

import numpy as np

_B, _LC, _LQ, _D = 8, 2048, 512, 512
_P = 128


def _ensure_import():
    try:
        import concourse.bass  # noqa: F401
    except ImportError:
        import sys

        for p in ("/opt/trn_rl_repo", "/root/.axon_site/_ro/trn_rl_repo"):
            if p not in sys.path:
                sys.path.insert(0, p)
        import concourse.bass  # noqa: F401


def build_program(Lc=_LC, Lq=_LQ, D=_D):
    """Build the single-core Bass program (identical across the 8 cores)."""
    _ensure_import()
    from contextlib import ExitStack

    import concourse.mybir as mybir
    from concourse import bacc
    from concourse import masks
    from concourse.tile import TileContext

    f32 = mybir.dt.float32
    f32r = mybir.dt.float32r
    bf16 = mybir.dt.bfloat16
    EXP = mybir.ActivationFunctionType.Exp
    AXX = mybir.AxisListType.X
    P = _P
    NLc, NLq, ND = Lc // P, Lq // P, D // P
    CHUNK = 512
    NCH = Lc // CHUNK
    WT = 4  # row-tiles per transpose window
    NW = NLc // WT

    nc = bacc.Bacc()
    dCT = nc.declare_dram_parameter("CTp", [P, NCH * ND * CHUNK], bf16, isOutput=False)
    dQmT = nc.declare_dram_parameter("QmTp", [P, ND * Lq], bf16, isOutput=False)
    dCbf = nc.declare_dram_parameter("Cbf", [Lc, D], bf16, isOutput=False)
    dQbf = nc.declare_dram_parameter("Qbfp", [P, NLq * D], bf16, isOutput=False)
    dccols = nc.declare_dram_parameter("c_cols", [P, NLc], f32, isOutput=False)
    deqcf = nc.declare_dram_parameter("eqcf", [P, NLq], f32, isOutput=False)
    dsm = nc.declare_dram_parameter("smalls", [1, Lq + P], f32r, isOutput=False)
    dout = nc.declare_dram_parameter("out", [Lc, 4 * D], bf16, isOutput=True)

    with ExitStack() as ctx:
        tc = ctx.enter_context(TileContext(nc))
        sb = ctx.enter_context(tc.tile_pool(name="persist", bufs=1))
        psum = ctx.enter_context(tc.tile_pool(name="psum", bufs=1, space="PSUM"))
        stage = ctx.enter_context(tc.tile_pool(name="stage", bufs=4))

        # ---- persistent SBUF tiles ----
        tCTn = [
            sb.tile([P, ND * CHUNK], bf16, tag=f"CTn{n}", name=f"CTn{n}")
            for n in range(NCH)
        ]
        tQmTs = sb.tile([P, ND * Lq], bf16, name="QmTs")
        tQmT = [tQmTs[:, k * Lq : (k + 1) * Lq] for k in range(ND)]
        tCb = [sb.tile([P, D], bf16, tag=f"Cb{i}", name=f"Cb{i}") for i in range(NLc)]
        tQs = sb.tile([P, NLq * D], bf16, name="Qs")
        tQ = [tQs[:, j * D : (j + 1) * D] for j in range(NLq)]
        tEn = [sb.tile([P, Lq], bf16, tag=f"En{i}", name=f"En{i}") for i in range(NLc)]
        tEt = [sb.tile([P, Lc], bf16, tag=f"Et{j}", name=f"Et{j}") for j in range(NLq)]
        tSc = [sb.tile([P, D], bf16, tag=f"Sc{j}", name=f"Sc{j}") for j in range(NLq)]
        tcb = sb.tile([P, NLc], f32, name="cbias")
        teqcf = sb.tile([P, NLq], f32, name="eqcf")
        tsm = sb.tile([1, Lq + P], f32r, name="tsm")
        tqrow = tsm[:, 0:Lq]
        tones = tsm[:, Lq : Lq + P]
        tident = sb.tile([P, P], bf16, name="ident")
        trs0 = [sb.tile([P, 1], f32, tag=f"rs0{i}", name=f"rs0{i}") for i in range(NLc)]
        trsr = [sb.tile([P, 1], f32, tag=f"rsr{i}", name=f"rsr{i}") for i in range(NLc)]
        tcsp = [sb.tile([P, NW], f32, tag=f"csp{j}", name=f"csp{j}") for j in range(NLq)]
        tcs0 = [sb.tile([P, 1], f32, tag=f"cs0{j}", name=f"cs0{j}") for j in range(NLq)]
        tcsr = [sb.tile([P, 1], f32, tag=f"csr{j}", name=f"csr{j}") for j in range(NLq)]

        twarm = sb.tile([P, Lq], bf16, name="twarm")
        texpqbB = sb.tile([P, Lq], bf16, name="expqbB")
        tscr = sb.tile([P, Lq], bf16, name="rs_scratch")

        # ---- input DMA (ordered by first-consumer time) ----
        # P1(0) needs CT[k][0]+QmT[k] in k order -- those go absolutely
        # first so the PE starts ASAP (each descriptor has ~0.7us latency).
        nc.gpsimd.memset(twarm[:], 0.25)
        masks.make_identity(nc, tident[:])
        # dummy exp: loads the scalar engine's activation table during the
        # DMA head instead of stalling the first real P2
        nc.scalar.activation(tcs0[0][:], twarm[:, 0:1], EXP)
        CW = ND * CHUNK
        nc.sync.dma_start(out=tCTn[0][:], in_=dCT[:, 0:CW])
        nc.sync.dma_start(out=tQmTs[:], in_=dQmT[:, :])
        nc.sync.dma_start(out=tsm[:], in_=dsm[:, :])
        nc.sync.dma_start(out=tcb[:], in_=dccols[:, :])
        nc.sync.dma_start(out=teqcf[:], in_=deqcf[:, :])
        nc.sync.dma_start(out=tCTn[1][:], in_=dCT[:, CW : 2 * CW])
        # Q (needed by P6 of window 0), then first window's Cbf, then the
        # rest of CT paced ahead of the P1 consumer, then remaining Cbf.
        nc.sync.dma_start(out=tQs[:], in_=dQbf[:, :])
        for i in range(WT):
            nc.sync.dma_start(out=tCb[i][:], in_=dCbf[i * P : (i + 1) * P, :])
        nc.sync.dma_start(out=tCTn[2][:], in_=dCT[:, 2 * CW : 3 * CW])
        for i in range(WT, 2 * WT):
            nc.sync.dma_start(out=tCb[i][:], in_=dCbf[i * P : (i + 1) * P, :])
        nc.sync.dma_start(out=tCTn[3][:], in_=dCT[:, 3 * CW : 4 * CW])
        for i in range(2 * WT, NLc):
            nc.sync.dma_start(out=tCb[i][:], in_=dCbf[i * P : (i + 1) * P, :])

        # ---- PE warmup: full-K (128-row) matmuls on a memset tile (no DMA
        # dependency, starts during the input-DMA head).  Real array
        # activity for the HAM clock-gate; sized to end about when P1(0)'s
        # operands have landed so P1 is never delayed.
        warm_ps = psum.tile([P, Lq], f32, tag="psA", name="warm_ps", bufs=2)
        for _w in range(12):
            nc.tensor.matmul(
                warm_ps[:], twarm[:, 0:P], twarm[:], start=True, stop=True
            )

        # ---- broadcast exp(qb) row across partitions (K=1 matmul) ----
        psQB = psum.tile([P, Lq], f32, tag="psA", name="psQB", bufs=2)
        nc.tensor.matmul(psQB[:], tones, tqrow, start=True, stop=True)
        nc.vector.tensor_copy(out=texpqbB[:], in_=psQB[:])

        # ---- main pipeline ----
        # per step s: P1/P2 for tile s; transposes for tile s-1 lag one step
        # behind so the scalar exp has a P1 of slack.  When a window's last
        # tile is transposed, the vector engine flushes the psT pair into
        # Et[j] (tensor_tensor_reduce bypass; accum -> colsum partials) and
        # P6 for that window runs one step later, giving the copies a P1+T
        # of slack.  Elementwise: A-scale on gpsimd, C*A on vector.
        cur_psT = None
        MUL = mybir.AluOpType.mult
        ADD = mybir.AluOpType.add

        def emit_copies_recips(w):
            # copies first: they gate P6 (PE); recips only gate A-scales
            for j in range(NLq):
                src = cur_psT[j // 2][:, (j % 2) * (WT * P) : (j % 2 + 1) * (WT * P)]
                nc.vector.tensor_scalar(
                    out=tEt[j][:, w * (WT * P) : (w + 1) * (WT * P)],
                    in0=src,
                    scalar1=teqcf[:, j : j + 1],
                    scalar2=None,
                    op0=MUL,
                    op1=ADD,
                    accum_out=tcsp[j][:, w : w + 1],
                )
            for i in range(w * WT, (w + 1) * WT):
                nc.vector.reciprocal(trsr[i][:], trs0[i][:])

        def emit_P6_tile(i):
            psA = psum.tile([P, D], f32, tag="psA", name=f"psa{i}", bufs=2)
            for j in range(NLq):
                nc.tensor.matmul(
                    psA[:],
                    tEt[j][:, i * P : (i + 1) * P],
                    tQ[j],
                    start=(j == 0),
                    stop=(j == NLq - 1),
                )
            tA = stage.tile([P, D], bf16, tag="A", name=f"A{i}")
            nc.scalar.mul(tA[:], psA[:], trsr[i][:])
            tCA = stage.tile([P, D], bf16, tag="CA", name=f"CA{i}")
            nc.gpsimd.tensor_mul(tCA[:], tCb[i][:], tA[:])
            nc.sync.dma_start(out=dout[i * P : (i + 1) * P, 0:D], in_=tCb[i][:])
            nc.sync.dma_start(out=dout[i * P : (i + 1) * P, D : 2 * D], in_=tA[:])
            nc.sync.dma_start(out=dout[i * P : (i + 1) * P, 2 * D : 3 * D], in_=tCA[:])

        psF = [None] * NLq

        def emit_P5_mms(j):
            psF[j] = psum.tile([P, D], f32, tag="ps", name=f"psf{j}", bufs=2)
            for k in range(NLc):
                nc.tensor.matmul(
                    psF[j][:],
                    tEn[k][:, j * P : (j + 1) * P],
                    tCb[k][:],
                    start=(k == 0),
                    stop=(k == NLc - 1),
                )

        def emit_P5_scale(j):
            if j == 0:
                for jj in range(NLq):
                    nc.vector.reduce_sum(tcs0[jj][:], tcsp[jj][:], axis=AXX)
                    nc.vector.reciprocal(tcsr[jj][:], tcs0[jj][:])
                    nc.vector.tensor_mul(
                        tcs0[jj][:], tcsr[jj][:], teqcf[:, jj : jj + 1]
                    )
            nc.vector.tensor_scalar_mul(tSc[j][:], psF[j][:], tcs0[j][:])

        for s in range(NLc + 5):
            if s < NLc:
                ps = psum.tile([P, Lq], f32, tag="ps", name=f"psn{s}", bufs=2)
                for k in range(ND):
                    nc.tensor.matmul(
                        ps[:],
                        tCTn[s // 4][
                            :, k * CHUNK + (s % 4) * P : k * CHUNK + (s % 4 + 1) * P
                        ],
                        tQmT[k],
                        start=(k == 0),
                        stop=(k == ND - 1),
                    )
                nc.scalar.activation(tEn[s][:], ps[:], EXP, bias=tcb[:, s : s + 1])
                nc.vector.tensor_mul(tscr[:], tEn[s][:], texpqbB[:])
                nc.vector.reduce_sum(trs0[s][:], tscr[:], axis=AXX)
            if s == NLc:
                # P5(j0) fills the PE between the last P1 and T(15): its
                # early k-matmuls depend only on long-finished exp outputs,
                # giving exp(15) time to land before T(15) consumes it.
                emit_P5_mms(0)
            if 1 <= s <= NLc:
                i = s - 1
                if i % WT == 0:
                    cur_psT = [
                        psum.tile(
                            [P, 2 * WT * P],
                            bf16,
                            tag="psT",
                            name=f"psT{i // WT}_{pr}",
                            bufs=4,
                        )
                        for pr in range(2)
                    ]
                for j in range(NLq):
                    nc.tensor.transpose(
                        cur_psT[j // 2][
                            :,
                            (j % 2) * (WT * P)
                            + (i % WT) * P : (j % 2) * (WT * P)
                            + (i % WT + 1) * P,
                        ],
                        tEn[i][:, j * P : (j + 1) * P],
                        tident[:],
                    )
                if i % WT == WT - 1:
                    emit_copies_recips(i // WT)
            # tail: P5(j) interleaved with the remaining P6 tiles so the
            # vector-side colsum finalize + Sc scales hide under PE matmuls
            if NLc + 1 <= s <= NLc + 3:
                j = s - NLc
                emit_P5_scale(j - 1)
                emit_P5_mms(j)
            # P6 spread one tile per step: tile s-5 (its window's copies are
            # emitted at step 4w+4 <= s-1, so the vector flush has slack)
            if 5 <= s < NLc + 5:
                emit_P6_tile(s - 5)
        emit_P5_scale(NLq - 1)

        # ---- P7: Bmbar per row-tile -> Bm (scalar), C*Bm (gpsimd) ----
        for i in range(NLc):
            psB = psum.tile([P, D], f32, tag="psA", name=f"psb{i}", bufs=2)
            for j in range(NLq):
                nc.tensor.matmul(
                    psB[:],
                    tEt[j][:, i * P : (i + 1) * P],
                    tSc[j][:],
                    start=(j == 0),
                    stop=(j == NLq - 1),
                )
            tBm = stage.tile([P, D], bf16, tag="BM", name=f"Bm{i}")
            nc.scalar.mul(tBm[:], psB[:], trsr[i][:])
            tCB = stage.tile([P, D], bf16, tag="CB", name=f"CB{i}")
            nc.vector.tensor_mul(tCB[:], tCb[i][:], tBm[:])
            nc.sync.dma_start(out=dout[i * P : (i + 1) * P, 3 * D : 4 * D], in_=tCB[:])

    nc.finalize()
    return nc


def round_fp32r(a):
    """Round fp32 to the fp32r encoding: RNE to 11 mantissa bits, low 12
    bits zero.  Matmul operands must carry this encoding (the PE consumes
    the top 20 bits)."""
    a = np.ascontiguousarray(a, np.float32)
    u = a.view(np.uint32)
    u = (u + 0x7FF + ((u >> 12) & 1)) & np.uint32(0xFFFFF000)
    return u.view(np.float32)


def pack_rows(a):
    """[R*128, N] -> [128, R*N]: 128-row bands concatenated along columns."""
    R = a.shape[0] // _P
    return np.ascontiguousarray(
        np.concatenate([a[r * _P : (r + 1) * _P, :] for r in range(R)], axis=1)
    )


def pack_ct(ct):
    """CT [D, Lc] -> [128, NCH*ND*CHUNK], chunk-major then k-band."""
    D, Lc = ct.shape
    CHUNK = 512
    blocks = []
    for n in range(Lc // CHUNK):
        for k in range(D // _P):
            blocks.append(ct[k * _P : (k + 1) * _P, n * CHUNK : (n + 1) * CHUNK])
    return np.ascontiguousarray(np.concatenate(blocks, axis=1))


def prepare_in_maps(C, Q, Wo_w, Wo_b):
    """Shard over batch; per batch precompute layouts + rank-1 vectors."""
    import ml_dtypes

    D = C.shape[-1]
    P = _P
    w = np.asarray(Wo_w, np.float32)[0]
    wc, wq, wm = w[:D], w[D : 2 * D], w[2 * D :]
    b0 = np.float32(np.asarray(Wo_b, np.float32)[0])
    in_maps = []
    for b in range(C.shape[0]):
        Cb = np.ascontiguousarray(C[b], np.float32)
        Qb = np.ascontiguousarray(Q[b], np.float32)
        cvec = (Cb @ wc).astype(np.float32)
        qbvec = (Qb @ wq + b0).astype(np.float32)
        in_maps.append(
            {
                "CTp": pack_ct(Cb.T.astype(ml_dtypes.bfloat16)),
                "Cbf": Cb.astype(ml_dtypes.bfloat16),
                "Qbfp": pack_rows(Qb.astype(ml_dtypes.bfloat16)),
                "QmTp": pack_rows((Qb * wm).T.astype(ml_dtypes.bfloat16)),
                "c_cols": np.ascontiguousarray(cvec.reshape(-1, _P).T),
                "eqcf": np.ascontiguousarray(np.exp(qbvec).reshape(-1, _P).T),
                "smalls": round_fp32r(
                    np.concatenate([np.exp(qbvec), np.ones(_P, np.float32)])[None, :]
                ),
            }
        )
    return in_maps


_prog_cache = {}


def _get_program():
    if "nc" not in _prog_cache:
        _prog_cache["nc"] = build_program()
    return _prog_cache["nc"]


def run(C, Q, Wo_w, Wo_b, **spmd_kwargs):
    """Run on hardware; returns (out [B,Lc,4d] f32, BassKernelResults)."""
    _ensure_import()
    from concourse.bass_utils import run_bass_kernel_spmd

    nc = _get_program()
    in_maps = prepare_in_maps(C, Q, Wo_w, Wo_b)
    res = run_bass_kernel_spmd(nc, in_maps, list(range(len(in_maps))), **spmd_kwargs)
    out = np.stack(
        [np.asarray(res.results[i]["out"], np.float32) for i in range(len(in_maps))],
        axis=0,
    )
    return out, res


def kernel(C, Q, Wo_w, Wo_b):
    out, _ = run(C, Q, Wo_w, Wo_b)
    return out


# revision 35
# speedup vs baseline: 1.1190x; 1.0128x over previous
"""CQAttention (context-query attention, BiDAF/QANet-style) Trainium2 kernel.

Problem: B=8, Lc=2048, Lq=512, d=512.
  S[b,i,j] = C_i.wc + Q_j.wq + sum_k wm_k C_ik Q_jk + b  (trilinear score)
  Sq = softmax_j(S); Sc = softmax_i(S)
  A  = Sq @ Q;  Bm = Sq @ (Sc^T @ C)
  out = [C | A | C*A | C*Bm]   -> [B, Lc, 4d]

Strategy: data-parallel over batch across the 8 NeuronCores (one batch per
core).  Per core:

  P1: S tile  = (C*wm) @ Q^T + qb aug-row      [128, Lq] PSUM   (f32r matmul)
  P2: E = exp(S + c_i)  (scalar engine; bias per-partition; accum -> rowsum)
  T : E^T via PE transpose (bf16, 1 cyc/row)   -> PSUM bf16
      scalar Copy PSUM->SBUF assembles Et[j] (accum -> colsum partials)
  P6: Abar = E @ Q      (lhsT = Et cols)   A = Abar * 1/rowsum
  P5: F = E^T @ C       (lhsT = En cols)   ScTC = F * 1/colsum
  P7: Bmbar = E @ ScTC  (lhsT = Et cols)   Bm = Bmbar * 1/rowsum

vs the previous version this drops the *recomputed* transposed-score matmul
(40960 PE cycles) in favor of 64 PE transposes of E (8192 cycles); the
transposed exp pass becomes a same-cost scalar Copy.  Output is written
bf16 (host upcasts) halving out-DMA to 8.4 MB; the redundant C f32 input
load is dropped (block0 passthrough + C*A / C*Bm read the bf16 copy).

Elementwise tail work is spread over three engines (A on vector, Bm-scale
on scalar, C*Bm on gpsimd) so the P7 tail stays PE-bound.

Host side precomputes cheap O(L*d) vectors and layout transposes:
  wc/wq/wm split, c = C@wc (col-bias), qb = Q@wq + bias (aug row),
  CT = C^T (f32r), QmT = (Q*wm)^T (f32r), Cbf/Qbf = bf16 casts.
"""

import numpy as np

_B, _LC, _LQ, _D = 8, 2048, 512, 512
_P = 128


def _ensure_import():
    try:
        import concourse.bass  # noqa: F401
    except ImportError:
        import sys

        for p in ("/opt/trn_rl_repo", "/root/.axon_site/_ro/trn_rl_repo"):
            if p not in sys.path:
                sys.path.insert(0, p)
        import concourse.bass  # noqa: F401


def build_program(Lc=_LC, Lq=_LQ, D=_D):
    """Build the single-core Bass program (identical across the 8 cores)."""
    _ensure_import()
    from contextlib import ExitStack

    import concourse.mybir as mybir
    from concourse import bacc
    from concourse import masks
    from concourse.tile import TileContext

    f32 = mybir.dt.float32
    f32r = mybir.dt.float32r
    bf16 = mybir.dt.bfloat16
    EXP = mybir.ActivationFunctionType.Exp
    AXX = mybir.AxisListType.X
    P = _P
    NLc, NLq, ND = Lc // P, Lq // P, D // P
    CHUNK = 512
    NCH = Lc // CHUNK
    WT = 4  # row-tiles per transpose window
    NW = NLc // WT

    nc = bacc.Bacc()
    dCT = nc.declare_dram_parameter("CTp", [P, NCH * ND * CHUNK], bf16, isOutput=False)
    dQmT = nc.declare_dram_parameter("QmTp", [P, ND * Lq], bf16, isOutput=False)
    dCbf = nc.declare_dram_parameter("Cbf", [Lc, D], bf16, isOutput=False)
    dQbf = nc.declare_dram_parameter("Qbfp", [P, NLq * D], bf16, isOutput=False)
    dccols = nc.declare_dram_parameter("c_cols", [P, NLc], f32, isOutput=False)
    deqcf = nc.declare_dram_parameter("eqcf", [P, NLq], f32, isOutput=False)
    dsm = nc.declare_dram_parameter("smalls", [1, Lq + P], f32r, isOutput=False)
    dout = nc.declare_dram_parameter("out", [Lc, 4 * D], bf16, isOutput=True)

    with ExitStack() as ctx:
        tc = ctx.enter_context(TileContext(nc))
        sb = ctx.enter_context(tc.tile_pool(name="persist", bufs=1))
        psum = ctx.enter_context(tc.tile_pool(name="psum", bufs=1, space="PSUM"))
        stage = ctx.enter_context(tc.tile_pool(name="stage", bufs=6))

        # ---- persistent SBUF tiles ----
        tCTn = [
            sb.tile([P, ND * CHUNK], bf16, tag=f"CTn{n}", name=f"CTn{n}")
            for n in range(NCH)
        ]
        tQmTs = sb.tile([P, ND * Lq], bf16, name="QmTs")
        tQmT = [tQmTs[:, k * Lq : (k + 1) * Lq] for k in range(ND)]
        tCb = [sb.tile([P, D], bf16, tag=f"Cb{i}", name=f"Cb{i}") for i in range(NLc)]
        tQs = sb.tile([P, NLq * D], bf16, name="Qs")
        tQ = [tQs[:, j * D : (j + 1) * D] for j in range(NLq)]
        tEn = [sb.tile([P, Lq], bf16, tag=f"En{i}", name=f"En{i}") for i in range(NLc)]
        tEt = [sb.tile([P, Lc], bf16, tag=f"Et{j}", name=f"Et{j}") for j in range(NLq)]
        tSc = [sb.tile([P, D], bf16, tag=f"Sc{j}", name=f"Sc{j}") for j in range(NLq)]
        tcb = sb.tile([P, NLc], f32, name="cbias")
        teqcf = sb.tile([P, NLq], f32, name="eqcf")
        tsm = sb.tile([1, Lq + P], f32r, name="tsm")
        tqrow = tsm[:, 0:Lq]
        tones = tsm[:, Lq : Lq + P]
        tident = sb.tile([P, P], bf16, name="ident")
        trs0 = [sb.tile([P, 1], f32, tag=f"rs0{i}", name=f"rs0{i}") for i in range(NLc)]
        trsr = [sb.tile([P, 1], f32, tag=f"rsr{i}", name=f"rsr{i}") for i in range(NLc)]
        tcsp = [sb.tile([P, NW], f32, tag=f"csp{j}", name=f"csp{j}") for j in range(NLq)]
        tcs0 = [sb.tile([P, 1], f32, tag=f"cs0{j}", name=f"cs0{j}") for j in range(NLq)]
        tcsr = [sb.tile([P, 1], f32, tag=f"csr{j}", name=f"csr{j}") for j in range(NLq)]

        twarm = sb.tile([P, Lq], bf16, name="twarm")
        texpqbB = sb.tile([P, Lq], bf16, name="expqbB")
        tscr = sb.tile([P, Lq], bf16, name="rs_scratch")

        # ---- input DMA (ordered by first-consumer time) ----
        # P1(0) needs CT[k][0]+QmT[k] in k order -- those go absolutely
        # first so the PE starts ASAP (each descriptor has ~0.7us latency).
        nc.gpsimd.memset(twarm[:], 0.25)
        masks.make_identity(nc, tident[:])
        # dummy exp: loads the scalar engine's activation table during the
        # DMA head instead of stalling the first real P2
        nc.scalar.activation(tcs0[0][:], twarm[:, 0:1], EXP)
        CW = ND * CHUNK
        nc.sync.dma_start(out=tCTn[0][:], in_=dCT[:, 0:CW])
        nc.sync.dma_start(out=tQmTs[:], in_=dQmT[:, :])
        nc.sync.dma_start(out=tsm[:], in_=dsm[:, :])
        nc.sync.dma_start(out=tcb[:], in_=dccols[:, :])
        nc.sync.dma_start(out=teqcf[:], in_=deqcf[:, :])
        nc.sync.dma_start(out=tCTn[1][:], in_=dCT[:, CW : 2 * CW])
        # Q (needed by P6 of window 0), then first window's Cbf, then the
        # rest of CT paced ahead of the P1 consumer, then remaining Cbf.
        nc.sync.dma_start(out=tQs[:], in_=dQbf[:, :])
        for i in range(WT):
            nc.sync.dma_start(out=tCb[i][:], in_=dCbf[i * P : (i + 1) * P, :])
        nc.sync.dma_start(out=tCTn[2][:], in_=dCT[:, 2 * CW : 3 * CW])
        for i in range(WT, 2 * WT):
            nc.sync.dma_start(out=tCb[i][:], in_=dCbf[i * P : (i + 1) * P, :])
        nc.sync.dma_start(out=tCTn[3][:], in_=dCT[:, 3 * CW : 4 * CW])
        for i in range(2 * WT, NLc):
            nc.sync.dma_start(out=tCb[i][:], in_=dCbf[i * P : (i + 1) * P, :])

        # ---- PE warmup: full-K (128-row) matmuls on a memset tile (no DMA
        # dependency, starts during the input-DMA head).  Real array
        # activity for the HAM clock-gate; sized to end about when P1(0)'s
        # operands have landed so P1 is never delayed.
        warm_ps = psum.tile([P, Lq], f32, tag="psA", name="warm_ps", bufs=2)
        for _w in range(12):
            nc.tensor.matmul(
                warm_ps[:], twarm[:, 0:P], twarm[:], start=True, stop=True
            )

        # ---- broadcast exp(qb) row across partitions (K=1 matmul) ----
        psQB = psum.tile([P, Lq], f32, tag="psA", name="psQB", bufs=2)
        nc.tensor.matmul(psQB[:], tones, tqrow, start=True, stop=True)
        nc.vector.tensor_copy(out=texpqbB[:], in_=psQB[:])

        # ---- main pipeline ----
        # per step s: P1/P2 for tile s; transposes for tile s-1 lag one step
        # behind so the scalar exp has a P1 of slack.  When a window's last
        # tile is transposed, the vector engine flushes the psT pair into
        # Et[j] (tensor_tensor_reduce bypass; accum -> colsum partials) and
        # P6 for that window runs one step later, giving the copies a P1+T
        # of slack.  Elementwise: A-scale on gpsimd, C*A on vector.
        cur_psT = None
        MUL = mybir.AluOpType.mult
        ADD = mybir.AluOpType.add

        def emit_copies_recips(w):
            # copies first: they gate P6 (PE); recips only gate A-scales
            for j in range(NLq):
                src = cur_psT[j // 2][:, (j % 2) * (WT * P) : (j % 2 + 1) * (WT * P)]
                nc.vector.tensor_scalar(
                    out=tEt[j][:, w * (WT * P) : (w + 1) * (WT * P)],
                    in0=src,
                    scalar1=teqcf[:, j : j + 1],
                    scalar2=None,
                    op0=MUL,
                    op1=ADD,
                    accum_out=tcsp[j][:, w : w + 1],
                )
            for i in range(w * WT, (w + 1) * WT):
                nc.vector.reciprocal(trsr[i][:], trs0[i][:])

        def emit_P6_tile(i):
            psA = psum.tile([P, D], f32, tag="psA", name=f"psa{i}", bufs=2)
            for j in range(NLq):
                nc.tensor.matmul(
                    psA[:],
                    tEt[j][:, i * P : (i + 1) * P],
                    tQ[j],
                    start=(j == 0),
                    stop=(j == NLq - 1),
                )
            tA = stage.tile([P, D], bf16, tag="A", name=f"A{i}")
            nc.scalar.mul(tA[:], psA[:], trsr[i][:])
            tCA = stage.tile([P, D], bf16, tag="CA", name=f"CA{i}")
            nc.gpsimd.tensor_mul(tCA[:], tCb[i][:], tA[:])
            nc.sync.dma_start(out=dout[i * P : (i + 1) * P, 0:D], in_=tCb[i][:])
            nc.sync.dma_start(out=dout[i * P : (i + 1) * P, D : 2 * D], in_=tA[:])
            nc.sync.dma_start(out=dout[i * P : (i + 1) * P, 2 * D : 3 * D], in_=tCA[:])

        psF = [None] * NLq

        def emit_P5_mms(j):
            psF[j] = psum.tile([P, D], f32, tag="ps", name=f"psf{j}", bufs=2)
            for k in range(NLc):
                nc.tensor.matmul(
                    psF[j][:],
                    tEn[k][:, j * P : (j + 1) * P],
                    tCb[k][:],
                    start=(k == 0),
                    stop=(k == NLc - 1),
                )

        def emit_P5_scale(j):
            if j == 0:
                for jj in range(NLq):
                    nc.vector.reduce_sum(tcs0[jj][:], tcsp[jj][:], axis=AXX)
                    nc.vector.reciprocal(tcsr[jj][:], tcs0[jj][:])
                    nc.vector.tensor_mul(
                        tcs0[jj][:], tcsr[jj][:], teqcf[:, jj : jj + 1]
                    )
            nc.vector.tensor_scalar_mul(tSc[j][:], psF[j][:], tcs0[j][:])

        for s in range(NLc + 6):
            if s < NLc:
                ps = psum.tile([P, Lq], f32, tag="ps", name=f"psn{s}", bufs=2)
                for k in range(ND):
                    nc.tensor.matmul(
                        ps[:],
                        tCTn[s // 4][
                            :, k * CHUNK + (s % 4) * P : k * CHUNK + (s % 4 + 1) * P
                        ],
                        tQmT[k],
                        start=(k == 0),
                        stop=(k == ND - 1),
                    )
                nc.scalar.activation(tEn[s][:], ps[:], EXP, bias=tcb[:, s : s + 1])
                nc.vector.tensor_mul(tscr[:], tEn[s][:], texpqbB[:])
                nc.vector.reduce_sum(trs0[s][:], tscr[:], axis=AXX)
            if s == NLc:
                # P5(j0) fills the PE between the last P1 and T(15): its
                # early k-matmuls depend only on long-finished exp outputs,
                # giving exp(15) time to land before T(15) consumes it.
                emit_P5_mms(0)
            if 1 <= s <= NLc:
                i = s - 1
                if i % WT == 0:
                    cur_psT = [
                        psum.tile(
                            [P, 2 * WT * P],
                            bf16,
                            tag="psT",
                            name=f"psT{i // WT}_{pr}",
                            bufs=4,
                        )
                        for pr in range(2)
                    ]
                for j in range(NLq):
                    nc.tensor.transpose(
                        cur_psT[j // 2][
                            :,
                            (j % 2) * (WT * P)
                            + (i % WT) * P : (j % 2) * (WT * P)
                            + (i % WT + 1) * P,
                        ],
                        tEn[i][:, j * P : (j + 1) * P],
                        tident[:],
                    )
                if i % WT == WT - 1:
                    emit_copies_recips(i // WT)
            # tail: P5(j) interleaved with the remaining P6 tiles so the
            # vector-side colsum finalize + Sc scales hide under PE matmuls
            if NLc + 1 <= s <= NLc + 3:
                j = s - NLc
                emit_P5_scale(j - 1)
                emit_P5_mms(j)
            # P6 spread one tile per step with two steps of lag behind the
            # window's Et flush so the vector copies never stall the PE
            if 6 <= s < NLc + 6:
                emit_P6_tile(s - 6)
        emit_P5_scale(NLq - 1)

        # ---- P7: Bmbar per row-tile -> Bm (scalar), C*Bm (gpsimd) ----
        for i in range(NLc):
            psB = psum.tile([P, D], f32, tag="psA", name=f"psb{i}", bufs=2)
            for j in range(NLq):
                nc.tensor.matmul(
                    psB[:],
                    tEt[j][:, i * P : (i + 1) * P],
                    tSc[j][:],
                    start=(j == 0),
                    stop=(j == NLq - 1),
                )
            tBm = stage.tile([P, D], bf16, tag="BM", name=f"Bm{i}")
            nc.scalar.mul(tBm[:], psB[:], trsr[i][:])
            tCB = stage.tile([P, D], bf16, tag="CB", name=f"CB{i}")
            nc.vector.tensor_mul(tCB[:], tCb[i][:], tBm[:])
            nc.sync.dma_start(out=dout[i * P : (i + 1) * P, 3 * D : 4 * D], in_=tCB[:])

    nc.finalize()
    return nc


def round_fp32r(a):
    """Round fp32 to the fp32r encoding: RNE to 11 mantissa bits, low 12
    bits zero.  Matmul operands must carry this encoding (the PE consumes
    the top 20 bits)."""
    a = np.ascontiguousarray(a, np.float32)
    u = a.view(np.uint32)
    u = (u + 0x7FF + ((u >> 12) & 1)) & np.uint32(0xFFFFF000)
    return u.view(np.float32)


def pack_rows(a):
    """[R*128, N] -> [128, R*N]: 128-row bands concatenated along columns."""
    R = a.shape[0] // _P
    return np.ascontiguousarray(
        np.concatenate([a[r * _P : (r + 1) * _P, :] for r in range(R)], axis=1)
    )


def pack_ct(ct):
    """CT [D, Lc] -> [128, NCH*ND*CHUNK], chunk-major then k-band."""
    D, Lc = ct.shape
    CHUNK = 512
    blocks = []
    for n in range(Lc // CHUNK):
        for k in range(D // _P):
            blocks.append(ct[k * _P : (k + 1) * _P, n * CHUNK : (n + 1) * CHUNK])
    return np.ascontiguousarray(np.concatenate(blocks, axis=1))


def prepare_in_maps(C, Q, Wo_w, Wo_b):
    """Shard over batch; per batch precompute layouts + rank-1 vectors."""
    import ml_dtypes

    D = C.shape[-1]
    P = _P
    w = np.asarray(Wo_w, np.float32)[0]
    wc, wq, wm = w[:D], w[D : 2 * D], w[2 * D :]
    b0 = np.float32(np.asarray(Wo_b, np.float32)[0])
    in_maps = []
    for b in range(C.shape[0]):
        Cb = np.ascontiguousarray(C[b], np.float32)
        Qb = np.ascontiguousarray(Q[b], np.float32)
        cvec = (Cb @ wc).astype(np.float32)
        qbvec = (Qb @ wq + b0).astype(np.float32)
        in_maps.append(
            {
                "CTp": pack_ct(Cb.T.astype(ml_dtypes.bfloat16)),
                "Cbf": Cb.astype(ml_dtypes.bfloat16),
                "Qbfp": pack_rows(Qb.astype(ml_dtypes.bfloat16)),
                "QmTp": pack_rows((Qb * wm).T.astype(ml_dtypes.bfloat16)),
                "c_cols": np.ascontiguousarray(cvec.reshape(-1, _P).T),
                "eqcf": np.ascontiguousarray(np.exp(qbvec).reshape(-1, _P).T),
                "smalls": round_fp32r(
                    np.concatenate([np.exp(qbvec), np.ones(_P, np.float32)])[None, :]
                ),
            }
        )
    return in_maps


_prog_cache = {}


def _get_program():
    if "nc" not in _prog_cache:
        _prog_cache["nc"] = build_program()
    return _prog_cache["nc"]


def run(C, Q, Wo_w, Wo_b, **spmd_kwargs):
    """Run on hardware; returns (out [B,Lc,4d] f32, BassKernelResults)."""
    _ensure_import()
    from concourse.bass_utils import run_bass_kernel_spmd

    nc = _get_program()
    in_maps = prepare_in_maps(C, Q, Wo_w, Wo_b)
    res = run_bass_kernel_spmd(nc, in_maps, list(range(len(in_maps))), **spmd_kwargs)
    out = np.stack(
        [np.asarray(res.results[i]["out"], np.float32) for i in range(len(in_maps))],
        axis=0,
    )
    return out, res


def kernel(C, Q, Wo_w, Wo_b):
    out, _ = run(C, Q, Wo_w, Wo_b)
    return out
